# revision 60
# baseline (speedup 1.0000x reference)
"""Trainium2 Bass kernel for nn_DiscreteLoss (data-parallel over batch).

Contract: kernel(**inputs) takes the FULL unsharded inputs (B=64) and
returns the FULL scalar loss.  Internally the batch dim is sharded over
8 NeuronCores (8 batches each); each core produces per-partition partial
sums for every loss term, which the host combines in float64.

Device-side strategy per core (8 batches, pipelined one batch at a
time so matmuls track DMA arrival):
  - bulk tensors ship as bf16, host-packed/transposed to [S, b, .]
    layouts and loaded with per-batch DMAs spread over both HWDGE
    rings (sync + scalar).
  - the mapping gather along S runs on the TensorEngine: a one-hot
    matrix E_b[j, s] = (mapping[b, s] == j) is built on-chip (PE row
    broadcast of the mapping values + DVE is_equal against an iota
    column); gather products are 1.0 * value, so bf16 matmul with fp32
    PSUM accumulation is exact over the (bf16-rounded) inputs.
  - the ground-truth subtraction is folded into the same PSUM
    accumulation group via a (-I) matmul (also exact).
  - host pre-scales rz/zs by 4 and masks by 1/16 (pure exponent shifts)
    so auto, disk, and seg share one normalizer: ONE ScalarE
    activation(Square, accum_out=...) pass reduces each batch.
  - landmark terms are tiny strided DVE reductions; the KL term is one
    Ln + one multiply + two reductions over all of qy.
  - partial sums are accumulated into two SBUF tiles (one per writing
    engine) and stored with two DMAs; the host combines in float64.
"""

import contextlib
import ctypes
import os
import sys
import types

for _p in ("/opt/trn_rl_repo", "/root/.axon_site/_ro/trn_rl_repo"):
    if os.path.isdir(_p) and _p not in sys.path:
        sys.path.append(_p)

import numpy as np

# --- problem constants (hardcoded per spec) ---
B, S, N, D, V = 64, 128, 128, 512, 128
N_CORES = 8
BPC = B // N_CORES          # batches per core = 8
GROUPS = 8                  # one group per batch: matmuls track DMA arrival
GSPEC = tuple((b, 1) for b in range(8))
GB = 2                      # max batches per group
ALPHA, BETA, GAMMA, EPS = 1.0, 0.1, 1.0, 1e-20
MARK = (0, 29, 88, 117)

_CACHE = {}


def _install_ntff_hook_shim():
    """run_bass_kernel_spmd(trace=True) looks for antenv.axon_hooks, which
    this image lacks; recreate the ctypes hook against libaxon_pjrt.so."""
    if "antenv.axon_hooks" in sys.modules:
        return
    so_path = "/opt/axon/libaxon_pjrt.so"

    def _get_hook():
        if not os.path.exists(so_path):
            return None
        lib = ctypes.CDLL(so_path)
        if not hasattr(lib, "axon_start_nrt_profile"):
            return None
        lib.axon_start_nrt_profile.argtypes = [
            ctypes.POINTER(ctypes.c_int64), ctypes.c_size_t]
        lib.axon_start_nrt_profile.restype = ctypes.c_int64
        lib.axon_stop_nrt_profile.argtypes = [ctypes.c_char_p]
        lib.axon_stop_nrt_profile.restype = ctypes.c_int64

        @contextlib.contextmanager
        def _hook(output_dir, device_ids):
            import jax
            jax.devices()
            if device_ids:
                ids = (ctypes.c_int64 * len(device_ids))(*device_ids)
                rc = lib.axon_start_nrt_profile(ids, len(device_ids))
            else:
                rc = lib.axon_start_nrt_profile(None, 0)
            if rc != 0:
                raise RuntimeError(f"axon_start_nrt_profile rc={rc}")
            try:
                yield
            finally:
                n = lib.axon_stop_nrt_profile(str(output_dir).encode())
                if n < 0:
                    raise RuntimeError(f"axon_stop_nrt_profile rc={n}")

        return _hook

    mod = types.ModuleType("antenv.axon_hooks")
    mod.get_axon_ntff_profile_hook = _get_hook
    mod.set_axon_ntff_profile_hook = lambda h: None
    sys.modules["antenv.axon_hooks"] = mod


def _build_program():
    import concourse.bacc as bacc
    import concourse.tile as tile
    from concourse import mybir

    f32 = mybir.dt.float32
    bf16 = mybir.dt.bfloat16
    nc = bacc.Bacc(None, target_bir_lowering=False, debug=False)

    # ---- per-core DRAM parameters (host pre-transposed/concatenated) ----
    # rzzs[s, b, 0:512] = rzs, [512:1024] = zs
    d_rzzs = nc.declare_dram_parameter("rzzs", [S, BPC, 2 * D], bf16, isOutput=False)
    # pmg[s, b, 0:256]=pts, [256:512]=masks, [512:768]=pts_gt, [768:1024]=masks_gt
    d_pmg = nc.declare_dram_parameter("pmg", [S, BPC, 4 * 2 * N], bf16, isOutput=False)
    d_qy = nc.declare_dram_parameter("qy", [S, BPC, V], bf16, isOutput=False)
    d_mapf = nc.declare_dram_parameter("mapf", [1, BPC * S], bf16, isOutput=False)
    # all six small "best" tensors packed to one [128, 128] block
    d_best = nc.declare_dram_parameter("best_all", [128, 128], f32, isOutput=False)
    # host-built constants
    d_iota = nc.declare_dram_parameter("iota", [128, 1], f32, isOutput=False)
    d_negi = nc.declare_dram_parameter("negI", [128, 128], bf16, isOutput=False)
    d_wbest = nc.declare_dram_parameter("wbest", [128, 16], f32, isOutput=False)
    d_wslice = nc.declare_dram_parameter("wslice", [128, 16], f32, isOutput=False)
    # outputs: per-partition partial sums
    d_oact = nc.declare_dram_parameter("o_act", [128, BPC + 4], f32, isOutput=True)
    d_odve = nc.declare_dram_parameter("o_dve", [128, 4 * BPC + 4], f32, isOutput=True)

    SQUARE = mybir.ActivationFunctionType.Square
    LN = mybir.ActivationFunctionType.Ln
    AL = mybir.AluOpType
    AX = mybir.AxisListType

    with tile.TileContext(nc) as tc:
        with contextlib.ExitStack() as ctx:
            singles = ctx.enter_context(tc.tile_pool(name="singles", bufs=1))
            pref = ctx.enter_context(tc.tile_pool(name="pref", bufs=1))
            data = ctx.enter_context(tc.tile_pool(name="data", bufs=2))
            junkp = ctx.enter_context(tc.tile_pool(name="junk", bufs=3))
            psp = ctx.enter_context(tc.tile_pool(name="ps", bufs=4, space="PSUM"))

            # ---- E-build inputs first: mapf gates the whole gather chain ----
            t_mapf = singles.tile([1, BPC * S], bf16)
            nc.sync.dma_start(out=t_mapf[:], in_=d_mapf.ap())
            t_ones = singles.tile([1, 128], bf16)
            nc.vector.memset(t_ones[:], 1.0)
            t_iota = singles.tile([128, 1], f32)
            nc.sync.dma_start(out=t_iota[:], in_=d_iota.ap())
            t_negi = singles.tile([128, 128], bf16)
            nc.sync.dma_start(out=t_negi[:], in_=d_negi.ap())
            t_eps = singles.tile([128, 1], f32)
            nc.vector.memset(t_eps[:], EPS)

            # ---- prefetch ALL group data up front (each group has own slot) ----
            rzzs_t, pmg_t = [], []
            qy_all = pref.tile([128, BPC * V], bf16, tag="qy")
            # three DMA dispatchers (2 HWDGE rings + SWDGE) round-robin per
            # batch: per-ring dispatch rate, not SDMA bandwidth, limits the
            # 2-ring configuration
            chans = (nc.sync, nc.scalar, nc.gpsimd)
            for g, (b0, nb) in enumerate(GSPEC):
                rzzs2 = pref.tile([128, nb, 2 * D], bf16, tag=f"rzzs{g}")
                chans[g % 3].dma_start(out=rzzs2[:], in_=d_rzzs.ap()[:, b0:b0 + nb, :])
                pmg2 = pref.tile([128, nb, 1024], bf16, tag=f"pmg{g}")
                chans[(g + 1) % 3].dma_start(out=pmg2[:], in_=d_pmg.ap()[:, b0:b0 + nb, :])
                if g == 1:
                    nc.sync.dma_start(out=qy_all[:], in_=d_qy.ap())
                rzzs_t.append(rzzs2); pmg_t.append(pmg2)

            t_wbest = singles.tile([128, 16], f32)
            nc.sync.dma_start(out=t_wbest[:], in_=d_wbest.ap())
            t_wslice = singles.tile([128, 16], f32)
            nc.sync.dma_start(out=t_wslice[:], in_=d_wslice.ap())

            # accumulators (each column written exactly once)
            a_act = singles.tile([128, BPC + 4], f32)
            a_dve = singles.tile([128, 4 * BPC + 4], f32)

            # ---- prologue: build all 8 one-hot matrices E_all[j, b*S+s] ----
            e_all = singles.tile([128, BPC * S], bf16)
            for h in range(2):
                ps_map = psp.tile([128, 512], f32, tag="m")
                nc.tensor.matmul(ps_map[:], lhsT=t_ones[:],
                                 rhs=t_mapf[:, h * 512:(h + 1) * 512],
                                 start=True, stop=True)
                nc.vector.tensor_scalar(
                    out=e_all[:, h * 512:(h + 1) * 512],
                    in0=ps_map[:],
                    scalar1=t_iota[:],
                    scalar2=None,
                    op0=AL.is_equal,
                )

            # ---- epilogue terms first so they overlap the main loop ----
            # best_all columns: [0:32]=best_rz [32:64]=logits [64:80]=best_pt
            # [80:96]=best_pt_gt [96:112]=best_mask [112:128]=best_mask_gt
            t_best = data.tile([128, 128], f32, tag="best")
            nc.scalar.dma_start(out=t_best[:], in_=d_best.ap())
            db = data.tile([128, 32], f32, tag="best32")
            nc.vector.tensor_sub(db[:], t_best[:, 0:32], t_best[:, 32:64])
            nc.scalar.activation(out=db[:], in_=db[:], func=SQUARE,
                                 accum_out=a_act[:, BPC + 0:BPC + 1])
            dp = data.tile([128, 16], f32, tag="best16")
            nc.vector.tensor_sub(dp[:], t_best[:, 64:80], t_best[:, 80:96])
            nc.scalar.activation(out=dp[:], in_=dp[:], func=SQUARE,
                                 accum_out=a_act[:, BPC + 1:BPC + 2])
            jb = junkp.tile([128, 16], f32, tag="jb")
            nc.vector.tensor_mul(jb[:], dp[:], t_wbest[:])
            nc.vector.tensor_reduce(out=a_act[:, BPC + 2:BPC + 3], in_=jb[:],
                                    axis=AX.X, op=AL.add)
            dm = data.tile([128, 16], f32, tag="best16")
            nc.vector.tensor_sub(dm[:], t_best[:, 96:112], t_best[:, 112:128])
            nc.scalar.activation(out=dm[:], in_=dm[:], func=SQUARE)
            jb2 = junkp.tile([128, 16], f32, tag="jb")
            nc.vector.tensor_mul(jb2[:], dm[:], t_wslice[:])
            nc.vector.tensor_reduce(out=a_act[:, BPC + 3:BPC + 4], in_=jb2[:],
                                    axis=AX.X, op=AL.add)

            # ---- KL block: one Ln over all of qy (single ACT table load) ----
            lnq = junkp.tile([128, BPC * V], f32, tag="lnq")
            nc.scalar.activation(out=lnq[:], in_=qy_all[:], func=LN, bias=t_eps[:])
            jkld = junkp.tile([128, BPC * V], f32, tag="jk")
            nc.vector.tensor_mul(jkld[:], qy_all[:], lnq[:])
            nc.vector.tensor_reduce(out=a_dve[:, 4 * BPC:4 * BPC + 1], in_=jkld[:], axis=AX.X, op=AL.add)
            nc.vector.tensor_reduce(out=a_dve[:, 4 * BPC + 1:4 * BPC + 2], in_=qy_all[:], axis=AX.X, op=AL.add)

            # ---- main loop per group (data already in flight) ----
            # rz/zs are pre-scaled by 4 on the host, so the auto sum shares
            # the disk/seg normalizer and ONE square+accum covers everything
            for g, (b0, nb) in enumerate(GSPEC):
                rzzs2, pmg2 = rzzs_t[g], pmg_t[g]
                pss = []
                # gather matmuls first (one E load per batch, two streams each)
                for b2 in range(nb):
                    b = b0 + b2
                    eb = e_all[:, b * S:(b + 1) * S]
                    ps = psp.tile([128, 1024], f32, tag="m")
                    pss.append(ps)
                    nc.tensor.matmul(ps[:, 0:512], lhsT=eb,
                                     rhs=rzzs2[:, b2, 0:512],
                                     start=True, stop=False)
                    nc.tensor.matmul(ps[:, 512:1024], lhsT=eb,
                                     rhs=pmg2[:, b2, 0:512],
                                     start=True, stop=False)
                # one negI load per group, 2*nb accumulate streams
                for b2 in range(nb):
                    nc.tensor.matmul(pss[b2][:, 0:512], lhsT=t_negi[:],
                                     rhs=rzzs2[:, b2, 512:1024],
                                     start=False, stop=True)
                    nc.tensor.matmul(pss[b2][:, 512:1024], lhsT=t_negi[:],
                                     rhs=pmg2[:, b2, 512:1024],
                                     start=False, stop=True)

                for b2 in range(nb):
                    b = b0 + b2
                    # one squared-sum per batch (to SBUF; frees PSUM at read).
                    # Final two batches run on the (then idle) DVE so the tail
                    # chain after the last matmul is short.
                    jsq = junkp.tile([128, 1024], f32, tag="jsq")
                    nc.scalar.activation(out=jsq[:], in_=pss[b2][:], func=SQUARE,
                                         accum_out=a_act[:, b:b + 1])
                    # landmark: 4 tiny strided reductions over squared pts part
                    for k, nk in enumerate(MARK):
                        nc.vector.tensor_reduce(
                            out=a_dve[:, 4 * b + k:4 * b + k + 1],
                            in_=jsq[:, 512 + 2 * nk:512 + 2 * nk + 2],
                            axis=AX.X, op=AL.add,
                        )

            # ---- store partials (one DMA per accumulator tile) ----
            nc.sync.dma_start(out=d_oact.ap(), in_=a_act[:])
            nc.scalar.dma_start(out=d_odve.ap(), in_=a_dve[:])

    nc.compile()
    return nc


def _get_program():
    if "nc" not in _CACHE:
        _CACHE["nc"] = _build_program()
    return _CACHE["nc"]


def _host_constants():
    iota = np.arange(128, dtype=np.float32).reshape(128, 1)
    # wbest / wslice over the host-flattened [BPC*N*2] -> [128, 16] layout
    wbest = np.zeros(BPC * N * 2, dtype=np.float32)
    wslice = np.zeros(BPC * N * 2, dtype=np.float32)
    for b in range(BPC):
        for n in MARK:
            wbest[b * 2 * N + 2 * n] = 1.0
            wbest[b * 2 * N + 2 * n + 1] = 1.0
        wslice[b * 2 * N + 2 * 32: b * 2 * N + 2 * 96] = 1.0
    import ml_dtypes
    return {
        "iota": iota,
        "negI": (-np.eye(128)).astype(ml_dtypes.bfloat16),
        "wbest": wbest.reshape(128, 16),
        "wslice": wslice.reshape(128, 16),
    }


def _shard_inputs(inputs):
    """Split the full B=64 inputs into 8 per-core input maps."""
    import ml_dtypes
    bf16 = ml_dtypes.bfloat16
    consts = _host_constants()
    f = lambda k: np.asarray(inputs[k], dtype=np.float32)
    # [B, S, X] views of everything, then one transpose+concat per pack
    rzzs = np.concatenate([f("rzs") * np.float32(4.0), f("zs") * np.float32(4.0)],
                          axis=2)                                          # [B,S,1024]
    # masks are pre-scaled by sqrt(1/(2N)) = 1/16 (exact in fp32) so the
    # seg sum folds into the disk accumulator with the right normalizer
    msc = np.float32(1.0 / 16.0)
    pmg = np.concatenate(
        [f("pts").reshape(B, S, 2 * N), f("masks").reshape(B, S, 2 * N) * msc,
         f("pts_gt").reshape(B, S, 2 * N), f("masks_gt").reshape(B, S, 2 * N) * msc,
         ], axis=2)                                                      # [B,S,1024]
    qy = f("qy")
    mapf = np.asarray(inputs["mapping"]).astype(np.float32)
    best_all = np.concatenate(
        [f("best_rz").reshape(N_CORES, 128, 32),
         f("logits").reshape(N_CORES, 128, 32),
         f("best_pt").reshape(N_CORES, 128, 16),
         f("best_pt_gt").reshape(N_CORES, 128, 16),
         f("best_mask").reshape(N_CORES, 128, 16),
         f("best_mask_gt").reshape(N_CORES, 128, 16)],
        axis=2)                                                          # [8,128,128]

    in_maps = []
    for c in range(N_CORES):
        lo, hi = c * BPC, (c + 1) * BPC
        m = {
            "rzzs": np.ascontiguousarray(rzzs[lo:hi].transpose(1, 0, 2)).astype(bf16),
            "pmg": np.ascontiguousarray(pmg[lo:hi].transpose(1, 0, 2)).astype(bf16),
            "qy": np.ascontiguousarray(qy[lo:hi].transpose(1, 0, 2)).astype(bf16),
            "mapf": np.ascontiguousarray(mapf[lo:hi].reshape(1, BPC * S)).astype(bf16),
            "best_all": np.ascontiguousarray(best_all[c]),
        }
        m.update(consts)
        in_maps.append(m)
    return in_maps


def _combine(results, ln_v):
    """Host-side float64 reduction of the per-core partial sums."""
    s_main = s_land = s_kld = s_qsum = 0.0
    s_best = np.zeros(4, dtype=np.float64)
    for r in results:
        oa = r["o_act"].astype(np.float64)
        od = r["o_dve"].astype(np.float64)
        s_main += oa[:, 0:BPC].sum()
        s_best += oa[:, BPC:BPC + 4].sum(axis=0)
        s_land += od[:, 0:4 * BPC].sum()
        s_kld += od[:, 4 * BPC].sum()
        s_qsum += od[:, 4 * BPC + 1].sum()
    s_kld = s_kld + ln_v * s_qsum

    # o_main = 16*S_auto + S_disk + S_seg/256, all over (B*S): equals
    # auto + disk + ALPHA*seg given the host pre-scales (x4 rz/zs, /16 masks)
    main = s_main / (B * S)
    land = s_land / (B * S)
    kld = s_kld / (B * S)
    best_auto = s_best[0] / (B * D)
    best_disk = s_best[1] / (B * N * 2) / (B * N)
    best_land = s_best[2] / (B * N)
    best_seg = s_best[3] / (B * 64 * 2)

    best_reg = best_disk + best_land
    ret = (GAMMA * (best_reg + best_auto + ALPHA * best_seg)
           + (main + land)
           + BETA * kld)
    return np.float32(ret * B)


def run_sharded(inputs, trace=False):
    """Compile (cached), run on the 8 cores, return (scalar, BassKernelResults)."""
    _install_ntff_hook_shim()
    from concourse.bass_utils import run_bass_kernel_spmd

    ln_v = float(np.log(float(inputs["vector_dims"])))
    nc = _get_program()
    in_maps = _shard_inputs(inputs)
    res = run_bass_kernel_spmd(nc, in_maps, list(range(N_CORES)), trace=trace)
    return _combine(res.results, ln_v), res


def kernel(**inputs) -> np.ndarray:
    out, _ = run_sharded(inputs, trace=False)
    return out


# revision 61
# speedup vs baseline: 1.0940x; 1.0940x over previous
"""Trainium2 Bass kernel for nn_DiscreteLoss (data-parallel over batch).

Contract: kernel(**inputs) takes the FULL unsharded inputs (B=64) and
returns the FULL scalar loss.  Internally the batch dim is sharded over
8 NeuronCores (8 batches each); each core produces per-partition partial
sums for every loss term, which the host combines in float64.

Device-side strategy per core (8 batches, pipelined one batch at a
time so matmuls track DMA arrival):
  - bulk tensors ship as bf16, host-packed/transposed to [S, b, .]
    layouts and loaded with per-batch DMAs spread over both HWDGE
    rings (sync + scalar).
  - the mapping gather along S runs on the TensorEngine: a one-hot
    matrix E_b[j, s] = (mapping[b, s] == j) is built on-chip (PE row
    broadcast of the mapping values + DVE is_equal against an iota
    column); gather products are 1.0 * value, so bf16 matmul with fp32
    PSUM accumulation is exact over the (bf16-rounded) inputs.
  - the ground-truth subtraction is folded into the same PSUM
    accumulation group via a (-I) matmul (also exact).
  - host pre-scales rz/zs by 4 and masks by 1/16 (pure exponent shifts)
    so auto, disk, and seg share one normalizer: ONE ScalarE
    activation(Square, accum_out=...) pass reduces each batch.
  - landmark terms are tiny strided DVE reductions; the KL term is one
    Ln + one multiply + two reductions over all of qy.
  - partial sums are accumulated into two SBUF tiles (one per writing
    engine) and stored with two DMAs; the host combines in float64.
"""

import contextlib
import ctypes
import os
import sys
import types

for _p in ("/opt/trn_rl_repo", "/root/.axon_site/_ro/trn_rl_repo"):
    if os.path.isdir(_p) and _p not in sys.path:
        sys.path.append(_p)

import numpy as np

# --- problem constants (hardcoded per spec) ---
B, S, N, D, V = 64, 128, 128, 512, 128
N_CORES = 8
BPC = B // N_CORES          # batches per core = 8
GROUPS = 8                  # one group per batch: matmuls track DMA arrival
GSPEC = tuple((b, 1) for b in range(8))
GB = 2                      # max batches per group
ALPHA, BETA, GAMMA, EPS = 1.0, 0.1, 1.0, 1e-20
MARK = (0, 29, 88, 117)

_CACHE = {}


def _install_ntff_hook_shim():
    """run_bass_kernel_spmd(trace=True) looks for antenv.axon_hooks, which
    this image lacks; recreate the ctypes hook against libaxon_pjrt.so."""
    if "antenv.axon_hooks" in sys.modules:
        return
    so_path = "/opt/axon/libaxon_pjrt.so"

    def _get_hook():
        if not os.path.exists(so_path):
            return None
        lib = ctypes.CDLL(so_path)
        if not hasattr(lib, "axon_start_nrt_profile"):
            return None
        lib.axon_start_nrt_profile.argtypes = [
            ctypes.POINTER(ctypes.c_int64), ctypes.c_size_t]
        lib.axon_start_nrt_profile.restype = ctypes.c_int64
        lib.axon_stop_nrt_profile.argtypes = [ctypes.c_char_p]
        lib.axon_stop_nrt_profile.restype = ctypes.c_int64

        @contextlib.contextmanager
        def _hook(output_dir, device_ids):
            import jax
            jax.devices()
            if device_ids:
                ids = (ctypes.c_int64 * len(device_ids))(*device_ids)
                rc = lib.axon_start_nrt_profile(ids, len(device_ids))
            else:
                rc = lib.axon_start_nrt_profile(None, 0)
            if rc != 0:
                raise RuntimeError(f"axon_start_nrt_profile rc={rc}")
            try:
                yield
            finally:
                n = lib.axon_stop_nrt_profile(str(output_dir).encode())
                if n < 0:
                    raise RuntimeError(f"axon_stop_nrt_profile rc={n}")

        return _hook

    mod = types.ModuleType("antenv.axon_hooks")
    mod.get_axon_ntff_profile_hook = _get_hook
    mod.set_axon_ntff_profile_hook = lambda h: None
    sys.modules["antenv.axon_hooks"] = mod


def _build_program():
    import concourse.bacc as bacc
    import concourse.tile as tile
    from concourse import mybir

    f32 = mybir.dt.float32
    bf16 = mybir.dt.bfloat16
    nc = bacc.Bacc(None, target_bir_lowering=False, debug=False)

    # ---- per-core DRAM parameters (host pre-transposed/concatenated) ----
    # rzzs[s, b, 0:512] = rzs, [512:1024] = zs
    d_rzzs = nc.declare_dram_parameter("rzzs", [S, BPC, 2 * D], bf16, isOutput=False)
    # pmg[s, b, 0:256]=pts, [256:512]=masks, [512:768]=pts_gt, [768:1024]=masks_gt
    d_pmg = nc.declare_dram_parameter("pmg", [S, BPC, 4 * 2 * N], bf16, isOutput=False)
    d_qy = nc.declare_dram_parameter("qy", [S, BPC, V], bf16, isOutput=False)
    d_mapf = nc.declare_dram_parameter("mapf", [1, BPC * S], bf16, isOutput=False)
    # all six small "best" tensors packed to one [128, 128] block
    d_best = nc.declare_dram_parameter("best_all", [128, 128], f32, isOutput=False)
    # host-built constants
    d_iota = nc.declare_dram_parameter("iota", [128, 1], f32, isOutput=False)
    d_negi = nc.declare_dram_parameter("negI", [128, 128], bf16, isOutput=False)
    d_wbest = nc.declare_dram_parameter("wbest", [128, 16], f32, isOutput=False)
    d_wslice = nc.declare_dram_parameter("wslice", [128, 16], f32, isOutput=False)
    # outputs: per-partition partial sums
    d_oact = nc.declare_dram_parameter("o_act", [128, BPC + 4], f32, isOutput=True)
    d_odve = nc.declare_dram_parameter("o_dve", [128, 4 * BPC + 4], f32, isOutput=True)

    SQUARE = mybir.ActivationFunctionType.Square
    LN = mybir.ActivationFunctionType.Ln
    AL = mybir.AluOpType
    AX = mybir.AxisListType

    with tile.TileContext(nc) as tc:
        with contextlib.ExitStack() as ctx:
            singles = ctx.enter_context(tc.tile_pool(name="singles", bufs=1))
            pref = ctx.enter_context(tc.tile_pool(name="pref", bufs=1))
            data = ctx.enter_context(tc.tile_pool(name="data", bufs=2))
            junkp = ctx.enter_context(tc.tile_pool(name="junk", bufs=3))
            psp = ctx.enter_context(tc.tile_pool(name="ps", bufs=4, space="PSUM"))

            # ---- E-build inputs first: mapf gates the whole gather chain ----
            t_mapf = singles.tile([1, BPC * S], bf16)
            nc.sync.dma_start(out=t_mapf[:], in_=d_mapf.ap())
            t_ones = singles.tile([1, 128], bf16)
            nc.vector.memset(t_ones[:], 1.0)
            t_iota = singles.tile([128, 1], f32)
            nc.sync.dma_start(out=t_iota[:], in_=d_iota.ap())
            t_negi = singles.tile([128, 128], bf16)
            nc.sync.dma_start(out=t_negi[:], in_=d_negi.ap())
            t_eps = singles.tile([128, 1], f32)
            nc.vector.memset(t_eps[:], EPS)

            # ---- prefetch ALL group data up front (each group has own slot) ----
            rzzs_t, pmg_t = [], []
            qy_all = pref.tile([128, BPC * V], bf16, tag="qy")
            for g, (b0, nb) in enumerate(GSPEC):
                rzzs2 = pref.tile([128, nb, 2 * D], bf16, tag=f"rzzs{g}")
                nc.sync.dma_start(out=rzzs2[:], in_=d_rzzs.ap()[:, b0:b0 + nb, :])
                pmg2 = pref.tile([128, nb, 1024], bf16, tag=f"pmg{g}")
                nc.scalar.dma_start(out=pmg2[:], in_=d_pmg.ap()[:, b0:b0 + nb, :])
                if g == 1:
                    nc.sync.dma_start(out=qy_all[:], in_=d_qy.ap())
                rzzs_t.append(rzzs2); pmg_t.append(pmg2)

            t_wbest = singles.tile([128, 16], f32)
            nc.sync.dma_start(out=t_wbest[:], in_=d_wbest.ap())
            t_wslice = singles.tile([128, 16], f32)
            nc.sync.dma_start(out=t_wslice[:], in_=d_wslice.ap())

            # accumulators (each column written exactly once)
            a_act = singles.tile([128, BPC + 4], f32)
            a_dve = singles.tile([128, 4 * BPC + 4], f32)

            # ---- prologue: build all 8 one-hot matrices E_all[j, b*S+s] ----
            e_all = singles.tile([128, BPC * S], bf16)
            for h in range(2):
                ps_map = psp.tile([128, 512], f32, tag="m")
                nc.tensor.matmul(ps_map[:], lhsT=t_ones[:],
                                 rhs=t_mapf[:, h * 512:(h + 1) * 512],
                                 start=True, stop=True)
                nc.vector.tensor_scalar(
                    out=e_all[:, h * 512:(h + 1) * 512],
                    in0=ps_map[:],
                    scalar1=t_iota[:],
                    scalar2=None,
                    op0=AL.is_equal,
                )

            # ---- epilogue terms first so they overlap the main loop ----
            # best_all columns: [0:32]=best_rz [32:64]=logits [64:80]=best_pt
            # [80:96]=best_pt_gt [96:112]=best_mask [112:128]=best_mask_gt
            t_best = data.tile([128, 128], f32, tag="best")
            nc.scalar.dma_start(out=t_best[:], in_=d_best.ap())
            db = data.tile([128, 32], f32, tag="best32")
            nc.vector.tensor_sub(db[:], t_best[:, 0:32], t_best[:, 32:64])
            nc.scalar.activation(out=db[:], in_=db[:], func=SQUARE,
                                 accum_out=a_act[:, BPC + 0:BPC + 1])
            dp = data.tile([128, 16], f32, tag="best16")
            nc.vector.tensor_sub(dp[:], t_best[:, 64:80], t_best[:, 80:96])
            nc.scalar.activation(out=dp[:], in_=dp[:], func=SQUARE,
                                 accum_out=a_act[:, BPC + 1:BPC + 2])
            jb = junkp.tile([128, 16], f32, tag="jb")
            nc.vector.tensor_mul(jb[:], dp[:], t_wbest[:])
            nc.vector.tensor_reduce(out=a_act[:, BPC + 2:BPC + 3], in_=jb[:],
                                    axis=AX.X, op=AL.add)
            dm = data.tile([128, 16], f32, tag="best16")
            nc.vector.tensor_sub(dm[:], t_best[:, 96:112], t_best[:, 112:128])
            nc.scalar.activation(out=dm[:], in_=dm[:], func=SQUARE)
            jb2 = junkp.tile([128, 16], f32, tag="jb")
            nc.vector.tensor_mul(jb2[:], dm[:], t_wslice[:])
            nc.vector.tensor_reduce(out=a_act[:, BPC + 3:BPC + 4], in_=jb2[:],
                                    axis=AX.X, op=AL.add)

            # ---- KL block: one Ln over all of qy (single ACT table load) ----
            lnq = junkp.tile([128, BPC * V], f32, tag="lnq")
            nc.scalar.activation(out=lnq[:], in_=qy_all[:], func=LN, bias=t_eps[:])
            jkld = junkp.tile([128, BPC * V], f32, tag="jk")
            nc.vector.tensor_mul(jkld[:], qy_all[:], lnq[:])
            nc.vector.tensor_reduce(out=a_dve[:, 4 * BPC:4 * BPC + 1], in_=jkld[:], axis=AX.X, op=AL.add)
            nc.vector.tensor_reduce(out=a_dve[:, 4 * BPC + 1:4 * BPC + 2], in_=qy_all[:], axis=AX.X, op=AL.add)

            # ---- main loop per group (data already in flight) ----
            # rz/zs are pre-scaled by 4 on the host, so the auto sum shares
            # the disk/seg normalizer and ONE square+accum covers everything
            for g, (b0, nb) in enumerate(GSPEC):
                rzzs2, pmg2 = rzzs_t[g], pmg_t[g]
                pss = []
                # gather matmuls first (one E load per batch, two streams each)
                for b2 in range(nb):
                    b = b0 + b2
                    eb = e_all[:, b * S:(b + 1) * S]
                    ps = psp.tile([128, 1024], f32, tag="m")
                    pss.append(ps)
                    nc.tensor.matmul(ps[:, 0:512], lhsT=eb,
                                     rhs=rzzs2[:, b2, 0:512],
                                     start=True, stop=False)
                    nc.tensor.matmul(ps[:, 512:1024], lhsT=eb,
                                     rhs=pmg2[:, b2, 0:512],
                                     start=True, stop=False)
                # one negI load per group, 2*nb accumulate streams
                for b2 in range(nb):
                    nc.tensor.matmul(pss[b2][:, 0:512], lhsT=t_negi[:],
                                     rhs=rzzs2[:, b2, 512:1024],
                                     start=False, stop=True)
                    nc.tensor.matmul(pss[b2][:, 512:1024], lhsT=t_negi[:],
                                     rhs=pmg2[:, b2, 512:1024],
                                     start=False, stop=True)

                for b2 in range(nb):
                    b = b0 + b2
                    # one squared-sum per batch (to SBUF; frees PSUM at read).
                    # Final two batches run on the (then idle) DVE so the tail
                    # chain after the last matmul is short.
                    jsq = junkp.tile([128, 1024], f32, tag="jsq")
                    nc.scalar.activation(out=jsq[:], in_=pss[b2][:], func=SQUARE,
                                         accum_out=a_act[:, b:b + 1])
                    # landmark: 4 tiny strided reductions over squared pts part
                    for k, nk in enumerate(MARK):
                        nc.vector.tensor_reduce(
                            out=a_dve[:, 4 * b + k:4 * b + k + 1],
                            in_=jsq[:, 512 + 2 * nk:512 + 2 * nk + 2],
                            axis=AX.X, op=AL.add,
                        )

            # ---- store partials (one DMA per accumulator tile) ----
            nc.sync.dma_start(out=d_oact.ap(), in_=a_act[:])
            nc.scalar.dma_start(out=d_odve.ap(), in_=a_dve[:])

    nc.compile()
    return nc


def _get_program():
    if "nc" not in _CACHE:
        _CACHE["nc"] = _build_program()
    return _CACHE["nc"]


def _host_constants():
    iota = np.arange(128, dtype=np.float32).reshape(128, 1)
    # wbest / wslice over the host-flattened [BPC*N*2] -> [128, 16] layout
    wbest = np.zeros(BPC * N * 2, dtype=np.float32)
    wslice = np.zeros(BPC * N * 2, dtype=np.float32)
    for b in range(BPC):
        for n in MARK:
            wbest[b * 2 * N + 2 * n] = 1.0
            wbest[b * 2 * N + 2 * n + 1] = 1.0
        wslice[b * 2 * N + 2 * 32: b * 2 * N + 2 * 96] = 1.0
    import ml_dtypes
    return {
        "iota": iota,
        "negI": (-np.eye(128)).astype(ml_dtypes.bfloat16),
        "wbest": wbest.reshape(128, 16),
        "wslice": wslice.reshape(128, 16),
    }


def _shard_inputs(inputs):
    """Split the full B=64 inputs into 8 per-core input maps."""
    import ml_dtypes
    bf16 = ml_dtypes.bfloat16
    consts = _host_constants()
    f = lambda k: np.asarray(inputs[k], dtype=np.float32)
    # [B, S, X] views of everything, then one transpose+concat per pack
    rzzs = np.concatenate([f("rzs") * np.float32(4.0), f("zs") * np.float32(4.0)],
                          axis=2)                                          # [B,S,1024]
    # masks are pre-scaled by sqrt(1/(2N)) = 1/16 (exact in fp32) so the
    # seg sum folds into the disk accumulator with the right normalizer
    msc = np.float32(1.0 / 16.0)
    pmg = np.concatenate(
        [f("pts").reshape(B, S, 2 * N), f("masks").reshape(B, S, 2 * N) * msc,
         f("pts_gt").reshape(B, S, 2 * N), f("masks_gt").reshape(B, S, 2 * N) * msc,
         ], axis=2)                                                      # [B,S,1024]
    qy = f("qy")
    mapf = np.asarray(inputs["mapping"]).astype(np.float32)
    best_all = np.concatenate(
        [f("best_rz").reshape(N_CORES, 128, 32),
         f("logits").reshape(N_CORES, 128, 32),
         f("best_pt").reshape(N_CORES, 128, 16),
         f("best_pt_gt").reshape(N_CORES, 128, 16),
         f("best_mask").reshape(N_CORES, 128, 16),
         f("best_mask_gt").reshape(N_CORES, 128, 16)],
        axis=2)                                                          # [8,128,128]

    in_maps = []
    for c in range(N_CORES):
        lo, hi = c * BPC, (c + 1) * BPC
        m = {
            "rzzs": np.ascontiguousarray(rzzs[lo:hi].transpose(1, 0, 2)).astype(bf16),
            "pmg": np.ascontiguousarray(pmg[lo:hi].transpose(1, 0, 2)).astype(bf16),
            "qy": np.ascontiguousarray(qy[lo:hi].transpose(1, 0, 2)).astype(bf16),
            "mapf": np.ascontiguousarray(mapf[lo:hi].reshape(1, BPC * S)).astype(bf16),
            "best_all": np.ascontiguousarray(best_all[c]),
        }
        m.update(consts)
        in_maps.append(m)
    return in_maps


def _combine(results, ln_v):
    """Host-side float64 reduction of the per-core partial sums."""
    s_main = s_land = s_kld = s_qsum = 0.0
    s_best = np.zeros(4, dtype=np.float64)
    for r in results:
        oa = r["o_act"].astype(np.float64)
        od = r["o_dve"].astype(np.float64)
        s_main += oa[:, 0:BPC].sum()
        s_best += oa[:, BPC:BPC + 4].sum(axis=0)
        s_land += od[:, 0:4 * BPC].sum()
        s_kld += od[:, 4 * BPC].sum()
        s_qsum += od[:, 4 * BPC + 1].sum()
    s_kld = s_kld + ln_v * s_qsum

    # o_main = 16*S_auto + S_disk + S_seg/256, all over (B*S): equals
    # auto + disk + ALPHA*seg given the host pre-scales (x4 rz/zs, /16 masks)
    main = s_main / (B * S)
    land = s_land / (B * S)
    kld = s_kld / (B * S)
    best_auto = s_best[0] / (B * D)
    best_disk = s_best[1] / (B * N * 2) / (B * N)
    best_land = s_best[2] / (B * N)
    best_seg = s_best[3] / (B * 64 * 2)

    best_reg = best_disk + best_land
    ret = (GAMMA * (best_reg + best_auto + ALPHA * best_seg)
           + (main + land)
           + BETA * kld)
    return np.float32(ret * B)


def run_sharded(inputs, trace=False):
    """Compile (cached), run on the 8 cores, return (scalar, BassKernelResults)."""
    _install_ntff_hook_shim()
    from concourse.bass_utils import run_bass_kernel_spmd

    ln_v = float(np.log(float(inputs["vector_dims"])))
    nc = _get_program()
    in_maps = _shard_inputs(inputs)
    res = run_bass_kernel_spmd(nc, in_maps, list(range(N_CORES)), trace=trace)
    return _combine(res.results, ln_v), res


def kernel(**inputs) -> np.ndarray:
    out, _ = run_sharded(inputs, trace=False)
    return out
